# revision 7
# baseline (speedup 1.0000x reference)
"""Trainium2 kernel for nn_Group_10: 3x3 replicate-pad conv [4,512,32,32] ->
[4,9728,32,32] (+bias) followed by a per-64-channel-chunk pixel shuffle to
[4,152,256,256].

Sharding: output channels across 8 cores (19 chunks of 64 = 1216 couts each,
padded to 1280 = 10 PE tiles of 128).

Layout plan (per 128-cout tile t, batch n):
  matmul psum[m, f]: partition m = [s4 s3 s2 s1 s0 e cb] (cout_in_tile =
  64*cb + 2*s + e), free f = [w2 w1 w0 h4..h0 w4 w3] (f = w2*512 +
  (w&3)*128 + h*4 + (w>>3)), produced by the same moving-operand AP as the
  direct conv.  The pixel shuffle maps (cc=2s+e, h, w) -> out[p, q] with
  p = s*8 + (w&7), q = e*128 + 4h + (w>>3).  Two DVE 32x32 stream
  transposes (with bit-permuting strided APs) rearrange to
  O2[P, x]: P = s*4 + ((w>>1)&3) = p>>1, x = cb*512 + (w&1)*256 + e*128 +
  4h + (w>>3) = (p&1)*256 + q, so each DRAM store is a clean
  [[512,128],[1,512]] pattern: 128-partition parallel, 2KB contiguous runs.
Matmuls run in bf16 (x and W host-cast; fp32 PSUM accumulation): the PE
streams bf16 at 1 cycle/row vs 2 for fp32r, halving tensor-engine time;
the accumulated bf16 quantization error (~2.4e-3 rel l2) is well inside
the 2e-2 gate.
"""

import numpy as np
from contextlib import ExitStack

import concourse.bass as bass
import concourse.mybir as mybir
import concourse.tile as tile
from concourse import bacc
from concourse.bass_utils import run_bass_kernel_spmd

F32 = mybir.dt.float32
BF16 = mybir.dt.bfloat16

N_CORES = 8
B = 4
CIN = 512
H = W_ = 32
COUT = 9728
NCHUNK = COUT // 64            # 152
CH_PER_CORE = NCHUNK // N_CORES  # 19
COUT_CORE = COUT // N_CORES    # 1216
NTILES = 10                    # 1216 padded to 1280 = 10 tiles of 128
HP = WP = 34                   # replicate-padded image
PIX = HP * WP                  # 1156
NCT = CIN // 128               # 4 cin tiles

# PE output-partition m = [s4 s3 s2 s1 s0 e cb] -> cout_in_tile = 64*cb+2*s+e
_m = np.arange(128)
COUT_IN_TILE = (64 * (_m & 1) + 2 * (_m >> 2) + ((_m >> 1) & 1)).astype(
    np.int64)  # [128]

_nc_cache = None


def _build_nc(rep=1, skip_mm=False, skip_out=False, skip_dve=False):
    """rep>1 wraps the body in an on-device For_i loop — identical I/O
    signature, used by test.py to measure per-iteration HW time by
    differencing wall-clocks against the rep=1 build.  skip_* are
    timing-diagnostic ablations (kernel() uses defaults)."""
    nc = bacc.Bacc("TRN2", target_bir_lowering=False, debug=False,
                   num_devices=N_CORES)

    xp = nc.dram_tensor("xp", [B, CIN, HP, WP], BF16, kind="ExternalInput")
    w = nc.dram_tensor("w", [NTILES, 128, NCT, 9, 128], BF16,
                       kind="ExternalInput")
    bias = nc.dram_tensor("bias", [128, NTILES], F32, kind="ExternalInput")
    out = nc.dram_tensor("out", [B, CH_PER_CORE, 256, 256], F32,
                         kind="ExternalOutput")

    with ExitStack() as ctx:
        tc = ctx.enter_context(tile.TileContext(nc))
        xpool = ctx.enter_context(tc.tile_pool(name="xpool", bufs=1))
        wpool = ctx.enter_context(tc.tile_pool(name="wpool", bufs=2))
        opool = ctx.enter_context(tc.tile_pool(name="opool", bufs=2))
        t1pool = ctx.enter_context(tc.tile_pool(name="t1pool", bufs=2))
        mpool = ctx.enter_context(tc.tile_pool(name="mpool", bufs=2))
        x2pool = ctx.enter_context(tc.tile_pool(name="x2pool", bufs=2))
        o2pool = ctx.enter_context(tc.tile_pool(name="o2pool", bufs=3))
        bpool = ctx.enter_context(tc.tile_pool(name="bpool", bufs=1))
        ppool = ctx.enter_context(tc.tile_pool(name="ppool", bufs=3,
                                               space="PSUM"))

        def body():
            # x resident in SBUF: partition = cin%128, free = (n, ct) slabs
            # of 1156 pixels in natural (h, w) padded order.
            x_sb = xpool.tile([128, B * NCT * PIX], BF16)
            xrow = x_sb.ap[0][0]
            xt = x_sb.tensor
            xoff0 = x_sb.offset

            def load_x(n, ct):
                dst = bass.AP(xt, xoff0 + (n * NCT + ct) * PIX,
                              [[xrow, 128], [1, PIX]])
                src = bass.AP(xp, (n * CIN + ct * 128) * PIX,
                              [[PIX, 128], [1, PIX]])
                nc.sync.dma_start(dst, src)

            # only n=0 slabs before the first weight tile; the rest are
            # issued right after w0's DMA so the first matmuls start early.
            for ct in range(NCT):
                load_x(0, ct)

            bias_sb = bpool.tile([128, NTILES], F32)

            def rhs_ap(n, ct, tap, bk):
                dy, dx = divmod(tap, 3)
                return bass.AP(
                    xt,
                    xoff0 + (n * NCT + ct) * PIX + dy * WP + dx + 4 * bk,
                    [[xrow, 128], [1, 4], [WP, 32], [8, 4]],
                )

            def store(t, n, psum):
                # bias add PSUM -> SBUF, write permutes natural
                # f = [w2 w1 w0 h4 h3 | h2 h1 h0 w4 w3] to
                # Phi = [w0 w2 w1 h4 h3 | h2 h1 h0 w4 w3]
                o_sb = opool.tile([128, 1024], F32)
                orow = o_sb.ap[0][0]
                prow = psum.ap[0][0]
                add_in = bass.AP(psum.tensor, psum.offset,
                                 [[prow, 128], [256, 4], [128, 2], [1, 128]])
                add_out = bass.AP(o_sb.tensor, o_sb.offset,
                                  [[orow, 128], [128, 4], [512, 2], [1, 128]])
                nc.vector.tensor_scalar_add(add_out, add_in,
                                            bias_sb[:, t:t + 1])
                if skip_dve:
                    return
                # T1 (plain 32x32 stream transpose): swap partition-low5
                # [s2 s1 s0 e cb] <-> free-low5 [h2 h1 h0 w4 w3]:
                # S' = [w0 w2 w1 h4 h3 | s2 s1 s0 e cb]
                t1_sb = t1pool.tile([128, 1024], F32)
                t1row = t1_sb.ap[0][0]
                nc.vector.transpose(t1_sb, o_sb)
                # psi_a: S' -> M = [h4 h3 s2 s1 s0 | cb | w0 w2 w1 | e]
                m_sb = mpool.tile([128, 1024], F32)
                mrow = m_sb.ap[0][0]
                pa_in = bass.AP(t1_sb.tensor, t1_sb.offset,
                                [[t1row, 128], [4, 32], [1, 2], [128, 8],
                                 [2, 2]])
                pa_out = bass.AP(m_sb.tensor, m_sb.offset,
                                 [[mrow, 128], [32, 32], [16, 2], [2, 8],
                                  [1, 2]])
                nc.vector.tensor_copy(pa_out, pa_in)
                # psi_b: M -> X = [cb w0 e h4 h3 | s2 s1 s0 w2 w1]
                x2_sb = x2pool.tile([128, 1024], F32)
                x2row = x2_sb.ap[0][0]
                pb_in = bass.AP(m_sb.tensor, m_sb.offset,
                                [[mrow, 128], [8, 4], [1, 2], [32, 32],
                                 [2, 4]])
                pb_out = bass.AP(x2_sb.tensor, x2_sb.offset,
                                 [[x2row, 128], [256, 4], [128, 2], [4, 32],
                                  [1, 4]])
                nc.vector.tensor_copy(pb_out, pb_in)
                # T2 (plain): swap partition-low5 [h2 h1 h0 w4 w3] <->
                # [s2 s1 s0 w2 w1] -> partition P = s*4 + ((w>>1)&3),
                # free F2 = [cb w0 e h4 h3 | h2 h1 h0 w4 w3] = cb*512 + p&1
                # *256 + q
                o2_sb = o2pool.tile([128, 1024], F32)
                o2row = o2_sb.ap[0][0]
                nc.vector.transpose(o2_sb, x2_sb)
                if skip_out:
                    return
                nchunks = 2 if t < NTILES - 1 else 1
                for cb in range(nchunks):
                    src = bass.AP(o2_sb.tensor, o2_sb.offset + cb * 512,
                                  [[o2row, 128], [1, 512]])
                    base = (n * CH_PER_CORE + 2 * t + cb) * 65536
                    dst = bass.AP(out, base, [[512, 128], [1, 512]])
                    nc.sync.dma_start(dst, src)

            w_ap = w[:]
            for t in range(NTILES):
                w_sb = wpool.tile([128, NCT * 9 * 128], BF16)
                nc.sync.dma_start(w_sb, w_ap[t])
                if t == 0:
                    for n in range(1, B):
                        for ct in range(NCT):
                            load_x(n, ct)
                    nc.sync.dma_start(bias_sb, bias[:])
                wrow = w_sb.ap[0][0]
                wt = w_sb.tensor
                woff = w_sb.offset

                def lhsT_ap(ct, tap):
                    return bass.AP(wt, woff + (ct * 9 + tap) * 128,
                                   [[wrow, 128], [1, 128]])

                for n in range(B):
                    psum = ppool.tile([128, 1024], F32)
                    if not skip_mm:
                        for bk in range(2):  # PSUM bank = w2
                            for tap in range(9):
                                for ct in range(NCT):
                                    nc.tensor.matmul(
                                        psum[:, 512 * bk:512 * (bk + 1)],
                                        lhsT_ap(ct, tap),
                                        rhs_ap(n, ct, tap, bk),
                                        start=(tap == 0 and ct == 0),
                                        stop=(tap == 8 and ct == 3),
                                    )
                    store(t, n, psum)

        if rep == 1:
            body()
        else:
            with tc.For_i(0, rep):
                body()

    nc.compile()
    return nc


def _host_prep(x, W, b):
    """Build per-core input maps."""
    bf16 = mybir.dt.np(mybir.dt.bfloat16)
    xpad = np.pad(np.asarray(x, dtype=np.float32),
                  ((0, 0), (0, 0), (1, 1), (1, 1)), mode="edge")
    xpad = np.ascontiguousarray(xpad.astype(bf16))
    W = np.asarray(W, dtype=np.float32)
    b = np.asarray(b, dtype=np.float32)

    in_maps = []
    for i in range(N_CORES):
        Ws = W[i * COUT_CORE:(i + 1) * COUT_CORE]          # [1216,512,3,3]
        Wp = np.zeros((NTILES * 128, CIN, 3, 3), np.float32)
        Wp[:COUT_CORE] = Ws
        gather = (np.arange(NTILES)[:, None] * 128 +
                  COUT_IN_TILE[None, :])                   # [10,128]
        Wg = Wp[gather]                                    # [10,128(m),512,3,3]
        Wg = Wg.reshape(NTILES, 128, NCT, 128, 9)          # [t,m,ct,p,tap]
        w_dev = np.ascontiguousarray(
            Wg.transpose(0, 3, 2, 4, 1).astype(bf16))  # [t,p,ct,tap,m]

        bp = np.zeros((NTILES * 128,), np.float32)
        bp[:COUT_CORE] = b[i * COUT_CORE:(i + 1) * COUT_CORE]
        bias_dev = np.ascontiguousarray(bp[gather].T)      # [128,10]

        in_maps.append({"xp": xpad, "w": w_dev, "bias": bias_dev})
    return in_maps


def _run(in_maps, trace=False):
    global _nc_cache
    if _nc_cache is None:
        _nc_cache = _build_nc()
    return run_bass_kernel_spmd(_nc_cache, in_maps,
                                core_ids=list(range(N_CORES)), trace=trace)


def kernel(x, W, b):
    in_maps = _host_prep(x, W, b)
    res = _run(in_maps)
    outs = [res.results[i]["out"] for i in range(N_CORES)]  # [4,19,256,256]
    full = np.concatenate(outs, axis=1)                     # [4,152,256,256]
    return full


# revision 9
# speedup vs baseline: 2.4703x; 2.4703x over previous
"""Trainium2 kernel for nn_Group_10: 3x3 replicate-pad conv [4,512,32,32] ->
[4,9728,32,32] (+bias) followed by a per-64-channel-chunk pixel shuffle to
[4,152,256,256].

Sharding: output channels across 8 cores (19 chunks of 64 = 1216 couts each,
padded to 1280 = 10 PE tiles of 128).

Layout plan (per 128-cout tile t, batch n):
  matmul psum[m, f]: partition m = [s4 s3 s2 s1 s0 e cb] (cout_in_tile =
  64*cb + 2*s + e), free f = [w2 w1 w0 h4..h0 w4 w3] (f = w2*512 +
  (w&3)*128 + h*4 + (w>>3)), produced by the same moving-operand AP as the
  direct conv.  The pixel shuffle maps (cc=2s+e, h, w) -> out[p, q] with
  p = s*8 + (w&7), q = e*128 + 4h + (w>>3).  Two DVE 32x32 stream
  transposes (with bit-permuting strided APs) rearrange to
  O2[P, x]: P = s*4 + ((w>>1)&3) = p>>1, x = cb*512 + (w&1)*256 + e*128 +
  4h + (w>>3) = (p&1)*256 + q, so each DRAM store is a clean
  [[512,128],[1,512]] pattern: 128-partition parallel, 2KB contiguous runs.
Matmuls run in bf16 (x and W host-cast; fp32 PSUM accumulation): the PE
streams bf16 at 1 cycle/row vs 2 for fp32r, halving tensor-engine time;
the accumulated bf16 quantization error (~2.4e-3 rel l2) is well inside
the 2e-2 gate.
"""

import numpy as np
from contextlib import ExitStack

import concourse.bass as bass
import concourse.mybir as mybir
import concourse.tile as tile
from concourse import bacc
from concourse.bass_utils import run_bass_kernel_spmd

F32 = mybir.dt.float32
BF16 = mybir.dt.bfloat16

N_CORES = 8
B = 4
CIN = 512
H = W_ = 32
COUT = 9728
NCHUNK = COUT // 64            # 152
CH_PER_CORE = NCHUNK // N_CORES  # 19
COUT_CORE = COUT // N_CORES    # 1216
NTILES = 10                    # 1216 padded to 1280 = 10 tiles of 128
HP = WP = 34                   # replicate-padded image
PIX = HP * WP                  # 1156
NCT = CIN // 128               # 4 cin tiles

# PE output-partition m = [s4 s3 s2 s1 s0 e cb] -> cout_in_tile = 64*cb+2*s+e
_m = np.arange(128)
COUT_IN_TILE = (64 * (_m & 1) + 2 * (_m >> 2) + ((_m >> 1) & 1)).astype(
    np.int64)  # [128]

_nc_cache = None


def _build_nc(rep=1, skip_mm=False, skip_out=False, skip_dve=False):
    """rep>1 wraps the body in an on-device For_i loop — identical I/O
    signature, used by test.py to measure per-iteration HW time by
    differencing wall-clocks against the rep=1 build.  skip_* are
    timing-diagnostic ablations (kernel() uses defaults)."""
    nc = bacc.Bacc("TRN2", target_bir_lowering=False, debug=False,
                   num_devices=N_CORES)

    xp = nc.dram_tensor("xp", [B, CIN, HP, WP], BF16, kind="ExternalInput")
    w = nc.dram_tensor("w", [NTILES, 128, NCT, 9, 128], BF16,
                       kind="ExternalInput")
    bias = nc.dram_tensor("bias", [128, NTILES], F32, kind="ExternalInput")
    out = nc.dram_tensor("out", [B, CH_PER_CORE, 256, 256], F32,
                         kind="ExternalOutput")

    with ExitStack() as ctx:
        tc = ctx.enter_context(tile.TileContext(nc))
        xpool = ctx.enter_context(tc.tile_pool(name="xpool", bufs=1))
        wpool = ctx.enter_context(tc.tile_pool(name="wpool", bufs=2))
        opool = ctx.enter_context(tc.tile_pool(name="opool", bufs=2))
        t1pool = ctx.enter_context(tc.tile_pool(name="t1pool", bufs=2))
        mpool = ctx.enter_context(tc.tile_pool(name="mpool", bufs=2))
        x2pool = ctx.enter_context(tc.tile_pool(name="x2pool", bufs=2))
        o2pool = ctx.enter_context(tc.tile_pool(name="o2pool", bufs=3))
        bpool = ctx.enter_context(tc.tile_pool(name="bpool", bufs=1))
        ppool = ctx.enter_context(tc.tile_pool(name="ppool", bufs=3,
                                               space="PSUM"))

        def body():
            # x resident in SBUF: partition = cin%128, free = (n, ct) slabs
            # of 1156 pixels in natural (h, w) padded order.
            x_sb = xpool.tile([128, B * NCT * PIX], BF16)
            xrow = x_sb.ap[0][0]
            xt = x_sb.tensor
            xoff0 = x_sb.offset

            def load_x(n, ct):
                dst = bass.AP(xt, xoff0 + (n * NCT + ct) * PIX,
                              [[xrow, 128], [1, PIX]])
                src = bass.AP(xp, (n * CIN + ct * 128) * PIX,
                              [[PIX, 128], [1, PIX]])
                nc.sync.dma_start(dst, src)

            # only n=0 slabs before the first weight tile; the rest are
            # issued right after w0's DMA so the first matmuls start early.
            for ct in range(NCT):
                load_x(0, ct)

            bias_sb = bpool.tile([128, NTILES], F32)

            def rhs_ap(n, ct, tap, bk):
                # raster pixel order f = h*32 + w (bank bit = h4): moving
                # operand streams contiguous 32-elem runs, which the PE
                # fetches at full rate (strided inner dims halve it).
                dy, dx = divmod(tap, 3)
                return bass.AP(
                    xt,
                    xoff0 + (n * NCT + ct) * PIX + (16 * bk + dy) * WP + dx,
                    [[xrow, 128], [WP, 16], [1, 32]],
                )

            def store(t, n, psum):
                # bias add PSUM -> SBUF, write permutes raster
                # f = [h4 h3 h2 h1 h0 w4 w3 w2 w1 w0] to
                # Phi = [w0 w2 w1 h4 h3 | h2 h1 h0 w4 w3]
                o_sb = opool.tile([128, 1024], F32)
                orow = o_sb.ap[0][0]
                prow = psum.ap[0][0]
                add_in = bass.AP(psum.tensor, psum.offset,
                                 [[prow, 128], [32, 32], [8, 4], [2, 4],
                                  [1, 2]])
                add_out = bass.AP(o_sb.tensor, o_sb.offset,
                                  [[orow, 128], [4, 32], [1, 4], [128, 4],
                                   [512, 2]])
                nc.vector.tensor_scalar_add(add_out, add_in,
                                            bias_sb[:, t:t + 1])
                if skip_dve:
                    return
                # T1 (plain 32x32 stream transpose): swap partition-low5
                # [s2 s1 s0 e cb] <-> free-low5 [h2 h1 h0 w4 w3]:
                # S' = [w0 w2 w1 h4 h3 | s2 s1 s0 e cb]
                t1_sb = t1pool.tile([128, 1024], F32)
                t1row = t1_sb.ap[0][0]
                nc.vector.transpose(t1_sb, o_sb)
                # psi_a: S' -> M = [h4 h3 s2 s1 s0 | cb | w0 w2 w1 | e]
                m_sb = mpool.tile([128, 1024], F32)
                mrow = m_sb.ap[0][0]
                pa_in = bass.AP(t1_sb.tensor, t1_sb.offset,
                                [[t1row, 128], [4, 32], [1, 2], [128, 8],
                                 [2, 2]])
                pa_out = bass.AP(m_sb.tensor, m_sb.offset,
                                 [[mrow, 128], [32, 32], [16, 2], [2, 8],
                                  [1, 2]])
                nc.vector.tensor_copy(pa_out, pa_in)
                # psi_b: M -> X = [cb w0 e h4 h3 | s2 s1 s0 w2 w1]
                x2_sb = x2pool.tile([128, 1024], F32)
                x2row = x2_sb.ap[0][0]
                pb_in = bass.AP(m_sb.tensor, m_sb.offset,
                                [[mrow, 128], [8, 4], [1, 2], [32, 32],
                                 [2, 4]])
                pb_out = bass.AP(x2_sb.tensor, x2_sb.offset,
                                 [[x2row, 128], [256, 4], [128, 2], [4, 32],
                                  [1, 4]])
                nc.vector.tensor_copy(pb_out, pb_in)
                # T2 (plain): swap partition-low5 [h2 h1 h0 w4 w3] <->
                # [s2 s1 s0 w2 w1] -> partition P = s*4 + ((w>>1)&3),
                # free F2 = [cb w0 e h4 h3 | h2 h1 h0 w4 w3] = cb*512 + p&1
                # *256 + q
                o2_sb = o2pool.tile([128, 1024], F32)
                o2row = o2_sb.ap[0][0]
                nc.vector.transpose(o2_sb, x2_sb)
                if skip_out:
                    return
                nchunks = 2 if t < NTILES - 1 else 1
                for cb in range(nchunks):
                    src = bass.AP(o2_sb.tensor, o2_sb.offset + cb * 512,
                                  [[o2row, 128], [1, 512]])
                    base = (n * CH_PER_CORE + 2 * t + cb) * 65536
                    dst = bass.AP(out, base, [[512, 128], [1, 512]])
                    nc.sync.dma_start(dst, src)

            w_ap = w[:]
            for t in range(NTILES):
                w_sb = wpool.tile([128, NCT * 9 * 128], BF16)
                nc.sync.dma_start(w_sb, w_ap[t])
                if t == 0:
                    for n in range(1, B):
                        for ct in range(NCT):
                            load_x(n, ct)
                    nc.sync.dma_start(bias_sb, bias[:])
                wrow = w_sb.ap[0][0]
                wt = w_sb.tensor
                woff = w_sb.offset

                def lhsT_ap(ct, tap):
                    return bass.AP(wt, woff + (ct * 9 + tap) * 128,
                                   [[wrow, 128], [1, 128]])

                for n in range(B):
                    psum = ppool.tile([128, 1024], F32)
                    if not skip_mm:
                        for bk in range(2):  # PSUM bank = w2
                            for tap in range(9):
                                for ct in range(NCT):
                                    nc.tensor.matmul(
                                        psum[:, 512 * bk:512 * (bk + 1)],
                                        lhsT_ap(ct, tap),
                                        rhs_ap(n, ct, tap, bk),
                                        start=(tap == 0 and ct == 0),
                                        stop=(tap == 8 and ct == 3),
                                    )
                    store(t, n, psum)

        if rep == 1:
            body()
        else:
            with tc.For_i(0, rep):
                body()

    nc.compile()
    return nc


def _host_prep(x, W, b):
    """Build per-core input maps."""
    bf16 = mybir.dt.np(mybir.dt.bfloat16)
    xpad = np.pad(np.asarray(x, dtype=np.float32),
                  ((0, 0), (0, 0), (1, 1), (1, 1)), mode="edge")
    xpad = np.ascontiguousarray(xpad.astype(bf16))
    W = np.asarray(W, dtype=np.float32)
    b = np.asarray(b, dtype=np.float32)

    in_maps = []
    for i in range(N_CORES):
        Ws = W[i * COUT_CORE:(i + 1) * COUT_CORE]          # [1216,512,3,3]
        Wp = np.zeros((NTILES * 128, CIN, 3, 3), np.float32)
        Wp[:COUT_CORE] = Ws
        gather = (np.arange(NTILES)[:, None] * 128 +
                  COUT_IN_TILE[None, :])                   # [10,128]
        Wg = Wp[gather]                                    # [10,128(m),512,3,3]
        Wg = Wg.reshape(NTILES, 128, NCT, 128, 9)          # [t,m,ct,p,tap]
        w_dev = np.ascontiguousarray(
            Wg.transpose(0, 3, 2, 4, 1).astype(bf16))  # [t,p,ct,tap,m]

        bp = np.zeros((NTILES * 128,), np.float32)
        bp[:COUT_CORE] = b[i * COUT_CORE:(i + 1) * COUT_CORE]
        bias_dev = np.ascontiguousarray(bp[gather].T)      # [128,10]

        in_maps.append({"xp": xpad, "w": w_dev, "bias": bias_dev})
    return in_maps


def _run(in_maps, trace=False):
    global _nc_cache
    if _nc_cache is None:
        _nc_cache = _build_nc()
    return run_bass_kernel_spmd(_nc_cache, in_maps,
                                core_ids=list(range(N_CORES)), trace=trace)


def kernel(x, W, b):
    in_maps = _host_prep(x, W, b)
    res = _run(in_maps)
    outs = [res.results[i]["out"] for i in range(N_CORES)]  # [4,19,256,256]
    full = np.concatenate(outs, axis=1)                     # [4,152,256,256]
    return full


# revision 11
# speedup vs baseline: 3.4264x; 1.3870x over previous
"""Trainium2 kernel for nn_Group_10: 3x3 replicate-pad conv [4,512,32,32] ->
[4,9728,32,32] (+bias) followed by a per-64-channel-chunk pixel shuffle to
[4,152,256,256].

Sharding: output channels across 8 cores (19 chunks of 64 = 1216 couts each,
padded to 1280 = 10 PE tiles of 128).

Layout plan (per 128-cout tile t, batch n):
  matmul psum[m, f]: partition m = [s4 s3 s2 s1 s0 e cb] (cout_in_tile =
  64*cb + 2*s + e), free f = [w2 w1 w0 h4..h0 w4 w3] (f = w2*512 +
  (w&3)*128 + h*4 + (w>>3)), produced by the same moving-operand AP as the
  direct conv.  The pixel shuffle maps (cc=2s+e, h, w) -> out[p, q] with
  p = s*8 + (w&7), q = e*128 + 4h + (w>>3).  Two DVE 32x32 stream
  transposes (with bit-permuting strided APs) rearrange to
  O2[P, x]: P = s*4 + ((w>>1)&3) = p>>1, x = cb*512 + (w&1)*256 + e*128 +
  4h + (w>>3) = (p&1)*256 + q, so each DRAM store is a clean
  [[512,128],[1,512]] pattern: 128-partition parallel, 2KB contiguous runs.
Matmuls run in bf16 (x and W host-cast; fp32 PSUM accumulation): the PE
streams bf16 at 1 cycle/row vs 2 for fp32r, halving tensor-engine time;
the accumulated bf16 quantization error (~2.4e-3 rel l2) is well inside
the 2e-2 gate.
"""

import numpy as np
from contextlib import ExitStack

import concourse.bass as bass
import concourse.mybir as mybir
import concourse.tile as tile
from concourse import bacc
from concourse.bass_utils import run_bass_kernel_spmd

F32 = mybir.dt.float32
BF16 = mybir.dt.bfloat16

N_CORES = 8
B = 4
CIN = 512
H = W_ = 32
COUT = 9728
NCHUNK = COUT // 64            # 152
CH_PER_CORE = NCHUNK // N_CORES  # 19
COUT_CORE = COUT // N_CORES    # 1216
NTILES = 10                    # 1216 padded to 1280 = 10 tiles of 128
HP = WP = 34                   # replicate-padded image
PIX = HP * WP                  # 1156
NCT = CIN // 128               # 4 cin tiles

# PE output-partition m = [s4 s3 s2 s1 s0 e cb] -> cout_in_tile = 64*cb+2*s+e
_m = np.arange(128)
COUT_IN_TILE = (64 * (_m & 1) + 2 * (_m >> 2) + ((_m >> 1) & 1)).astype(
    np.int64)  # [128]

_nc_cache = None


def _build_nc(rep=1, skip_mm=False, skip_out=False, skip_dve=False):
    """rep>1 wraps the body in an on-device For_i loop — identical I/O
    signature, used by test.py to measure per-iteration HW time by
    differencing wall-clocks against the rep=1 build.  skip_* are
    timing-diagnostic ablations (kernel() uses defaults)."""
    nc = bacc.Bacc("TRN2", target_bir_lowering=False, debug=False,
                   num_devices=N_CORES)

    xp = nc.dram_tensor("xp", [B, CIN, HP, WP], BF16, kind="ExternalInput")
    w = nc.dram_tensor("w", [NTILES, 128, NCT, 9, 128], BF16,
                       kind="ExternalInput")
    bias = nc.dram_tensor("bias", [128, NTILES], F32, kind="ExternalInput")
    out = nc.dram_tensor("out", [B, CH_PER_CORE, 256, 256], F32,
                         kind="ExternalOutput")

    with ExitStack() as ctx:
        tc = ctx.enter_context(tile.TileContext(nc))
        xpool = ctx.enter_context(tc.tile_pool(name="xpool", bufs=1))
        wpool = ctx.enter_context(tc.tile_pool(name="wpool", bufs=2))
        opool = ctx.enter_context(tc.tile_pool(name="opool", bufs=2))
        t1pool = ctx.enter_context(tc.tile_pool(name="t1pool", bufs=2))
        mpool = ctx.enter_context(tc.tile_pool(name="mpool", bufs=2))
        x2pool = ctx.enter_context(tc.tile_pool(name="x2pool", bufs=2))
        o2pool = ctx.enter_context(tc.tile_pool(name="o2pool", bufs=3))
        bpool = ctx.enter_context(tc.tile_pool(name="bpool", bufs=1))
        ppool = ctx.enter_context(tc.tile_pool(name="ppool", bufs=3,
                                               space="PSUM"))

        def body():
            # x resident in SBUF: partition = cin%128, free = (n, ct) slabs
            # of 1156 pixels in natural (h, w) padded order.
            x_sb = xpool.tile([128, B * NCT * PIX], BF16)
            xrow = x_sb.ap[0][0]
            xt = x_sb.tensor
            xoff0 = x_sb.offset

            def load_x(n, ct):
                dst = bass.AP(xt, xoff0 + (n * NCT + ct) * PIX,
                              [[xrow, 128], [1, PIX]])
                src = bass.AP(xp, (n * CIN + ct * 128) * PIX,
                              [[PIX, 128], [1, PIX]])
                nc.sync.dma_start(dst, src)

            # only n=0 slabs before the first weight tile; the rest are
            # issued right after w0's DMA so the first matmuls start early.
            for ct in range(NCT):
                load_x(0, ct)

            bias_sb = bpool.tile([128, NTILES], F32)

            def rhs_ap(n, ct, tap, bk):
                # raster pixel order f = h*32 + w (bank bit = h4): moving
                # operand streams contiguous 32-elem runs, which the PE
                # fetches at full rate (strided inner dims halve it).
                dy, dx = divmod(tap, 3)
                return bass.AP(
                    xt,
                    xoff0 + (n * NCT + ct) * PIX + (16 * bk + dy) * WP + dx,
                    [[xrow, 128], [WP, 16], [1, 32]],
                )

            def store(t, n, psum):
                # bias add PSUM -> SBUF, write permutes raster
                # f = [h4 h3 h2 h1 h0 w4 w3 w2 w1 w0] to
                # Phi = [w0 w2 w1 h4 h3 | h2 h1 h0 w4 w3]
                o_sb = opool.tile([128, 1024], F32)
                orow = o_sb.ap[0][0]
                prow = psum.ap[0][0]
                add_in = bass.AP(psum.tensor, psum.offset,
                                 [[prow, 128], [32, 32], [8, 4], [2, 4],
                                  [1, 2]])
                add_out = bass.AP(o_sb.tensor, o_sb.offset,
                                  [[orow, 128], [4, 32], [1, 4], [128, 4],
                                   [512, 2]])
                nc.vector.tensor_scalar_add(add_out, add_in,
                                            bias_sb[:, t:t + 1])
                if skip_dve:
                    return
                # T1 (plain 32x32 stream transpose): swap partition-low5
                # [s2 s1 s0 e cb] <-> free-low5 [h2 h1 h0 w4 w3]:
                # S' = [w0 w2 w1 h4 h3 | s2 s1 s0 e cb]
                t1_sb = t1pool.tile([128, 1024], F32)
                t1row = t1_sb.ap[0][0]
                nc.vector.transpose(t1_sb, o_sb)
                # psi_a: S' -> M = [h4 h3 s2 s1 s0 | cb | w0 w2 w1 | e]
                m_sb = mpool.tile([128, 1024], F32)
                mrow = m_sb.ap[0][0]
                pa_in = bass.AP(t1_sb.tensor, t1_sb.offset,
                                [[t1row, 128], [4, 32], [1, 2], [128, 8],
                                 [2, 2]])
                pa_out = bass.AP(m_sb.tensor, m_sb.offset,
                                 [[mrow, 128], [32, 32], [16, 2], [2, 8],
                                  [1, 2]])
                nc.vector.tensor_copy(pa_out, pa_in)
                # psi_b: M -> X = [cb w0 e h4 h3 | s2 s1 s0 w2 w1]
                x2_sb = x2pool.tile([128, 1024], F32)
                x2row = x2_sb.ap[0][0]
                pb_in = bass.AP(m_sb.tensor, m_sb.offset,
                                [[mrow, 128], [8, 4], [1, 2], [32, 32],
                                 [2, 4]])
                pb_out = bass.AP(x2_sb.tensor, x2_sb.offset,
                                 [[x2row, 128], [256, 4], [128, 2], [4, 32],
                                  [1, 4]])
                nc.vector.tensor_copy(pb_out, pb_in)
                # T2 (plain): swap partition-low5 [h2 h1 h0 w4 w3] <->
                # [s2 s1 s0 w2 w1] -> partition P = s*4 + ((w>>1)&3),
                # free F2 = [cb w0 e h4 h3 | h2 h1 h0 w4 w3] = cb*512 + p&1
                # *256 + q
                o2_sb = o2pool.tile([128, 1024], F32)
                o2row = o2_sb.ap[0][0]
                nc.vector.transpose(o2_sb, x2_sb)
                if skip_out:
                    return
                nchunks = 2 if t < NTILES - 1 else 1
                for cb in range(nchunks):
                    src = bass.AP(o2_sb.tensor, o2_sb.offset + cb * 512,
                                  [[o2row, 128], [1, 512]])
                    base = (n * CH_PER_CORE + 2 * t + cb) * 65536
                    dst = bass.AP(out, base, [[512, 128], [1, 512]])
                    nc.sync.dma_start(dst, src)

            w_ap = w[:]
            for t in range(NTILES):
                w_sb = wpool.tile([128, NCT * 9 * 128], BF16)
                nc.sync.dma_start(w_sb, w_ap[t])
                if t == 0:
                    for n in range(1, B):
                        for ct in range(NCT):
                            load_x(n, ct)
                    nc.sync.dma_start(bias_sb, bias[:])
                wrow = w_sb.ap[0][0]
                wt = w_sb.tensor
                woff = w_sb.offset

                def lhsT_ap(ct, tap):
                    return bass.AP(wt, woff + (ct * 9 + tap) * 128,
                                   [[wrow, 128], [1, 128]])

                for n in range(B):
                    psum = ppool.tile([128, 1024], F32)
                    if not skip_mm:
                        for bk in range(2):  # PSUM bank = w2
                            for tap in range(9):
                                for ct in range(NCT):
                                    nc.tensor.matmul(
                                        psum[:, 512 * bk:512 * (bk + 1)],
                                        lhsT_ap(ct, tap),
                                        rhs_ap(n, ct, tap, bk),
                                        start=(tap == 0 and ct == 0),
                                        stop=(tap == 8 and ct == 3),
                                    )
                    store(t, n, psum)

        if rep == 1:
            body()
        else:
            with tc.For_i(0, rep):
                body()

    nc.compile()
    return nc


def _host_prep(x, W, b):
    """Build per-core input maps."""
    bf16 = mybir.dt.np(mybir.dt.bfloat16)
    xpad = np.pad(np.asarray(x, dtype=np.float32),
                  ((0, 0), (0, 0), (1, 1), (1, 1)), mode="edge")
    xpad = np.ascontiguousarray(xpad.astype(bf16))
    W = np.asarray(W, dtype=np.float32)
    b = np.asarray(b, dtype=np.float32)

    in_maps = []
    for i in range(N_CORES):
        Ws = W[i * COUT_CORE:(i + 1) * COUT_CORE]          # [1216,512,3,3]
        Wp = np.zeros((NTILES * 128, CIN, 3, 3), np.float32)
        Wp[:COUT_CORE] = Ws
        gather = (np.arange(NTILES)[:, None] * 128 +
                  COUT_IN_TILE[None, :])                   # [10,128]
        Wg = Wp[gather]                                    # [10,128(m),512,3,3]
        Wg = Wg.reshape(NTILES, 128, NCT, 128, 9)          # [t,m,ct,p,tap]
        w_dev = np.ascontiguousarray(
            Wg.transpose(0, 3, 2, 4, 1).astype(bf16))  # [t,p,ct,tap,m]

        bp = np.zeros((NTILES * 128,), np.float32)
        bp[:COUT_CORE] = b[i * COUT_CORE:(i + 1) * COUT_CORE]
        bias_dev = np.ascontiguousarray(bp[gather].T)      # [128,10]

        in_maps.append({"xp": xpad, "w": w_dev, "bias": bias_dev})
    return in_maps


def _run(in_maps, trace=False):
    global _nc_cache
    if _nc_cache is None:
        _nc_cache = _build_nc()
    return run_bass_kernel_spmd(_nc_cache, in_maps,
                                core_ids=list(range(N_CORES)), trace=trace)


def kernel(x, W, b):
    in_maps = _host_prep(x, W, b)
    res = _run(in_maps)
    outs = [res.results[i]["out"] for i in range(N_CORES)]  # [4,19,256,256]
    full = np.concatenate(outs, axis=1)                     # [4,152,256,256]
    return full
